# revision 47
# baseline (speedup 1.0000x reference)
"""VQ codebook squared-distance kernel for Trainium2 (8 NeuronCores).

Computes dist[n,k,l] = (||x[n,:,l]||^2 + ||w[k,:]||^2 - 2*x[n,:,l].w[k,:]) / scale^2
for x (32,128,3136) f32, weight (64,128) f32, scale (1,) f32 -> out (32,64,3136) f32.

Sharding: data-parallel over N (4 per core); weight/scale replicated.

Design (best-measured configuration across 12 HW iterations: 36445 /
37040 ns vs the 37038 ns baseline):
  - Input stream is HBM-stack-roofline-bound (~343 GB/s/core with both
    NCs of a stack active): 6.42 MB f32 x read in ~18us. The kernel is
    a saturated multi-resource equilibrium: PE ~13-16us, ACT ~11-14us,
    DVE ~13us busy inside a ~22us window, plus a FIXED ~8.5us NEFF end
    block (barrier + 257 walrus per-semaphore clears, invariant to
    kernel content and not HAM-gated). The back half runs as a conveyor
    paced by ACT's epilogue rate (~480ns fixed cost per ACT
    instruction) through PSUM-pool recycling; shifting epilogue work
    between ACT and DVE moves the pacer but not the total (measured
    both ways).
  - x loads via SWDGE Q0 cast-on-load f32->f16 in graded pieces:
    small head (PE starts by ~11us), fine interleaved tail so each
    completion sem gates at most two chunks of matmuls. SWDGE
    straggler: one SDMA engine lags the other 15 by an amount that
    grows with Q0 descriptor pressure (~0 at 8 transfers, ~0.4-1.65us
    at 12, ~2.4-4us beyond, or with 3+ full-image transfers).
  - Chunks 6-7 of n2/n3 arrive via HWDGE (raw f32, ACT-cast to f16,
    DVE-squared straight from f32): lag-free sems, ready mid-stream.
    Never put casts on GpSimd - Pool tensor ops lock the shared
    DVE/GpSimd SBUF ports and knock DVE out of 2x perf mode.
  - Outputs ride HWDGE in readiness-ordered pieces (no Q0 descriptor
    traffic, no queueing behind the straggler): pair 0 as one full-L
    write, pair 1 as ch(6,8)/ch(0,3)/ch(3,5)/ch(5,6) with the final
    50 KB piece last. Epilogues split ACT || DVE for the last chunks
    (NOTE: the DVE half is still serialized after the ACT half by
    cross-engine write ordering on the shared out_t tile; a separate
    single-writer tile avoids that but measured no net win - the
    conveyor just finds another pacer).
  - Output is offset fp8: e4m3(dist - 2D/s^2), host adds the offset
    back. Centering removes the ~2D/s^2 common mode so e4m3's ~6%
    step applies to the +-170 residual only: rel_l2 ~3e-3 (vs 2e-2
    budget) for half the write traffic.
  - PE: psum = (-2Wt)f16 @ x_f16 + ones_f16 @ (x^2)_f16, two images
    per PSUM tile via column tiling (tile_position (0,0)/(0,64));
    the h0/h64 col-group matmul pairs run concurrently (~1.4ns/col).
  - scale broadcast 1->128 via 1-col fp32 matmul; weight transpose on
    PE (identity built early on gpsimd).
"""

import numpy as np

N, D, L, K = 32, 128, 3136, 64
N_CORES = 8
NS = N // N_CORES          # n's per core
LC = 392                   # matmul chunk (8 per image, one PSUM bank)
HC = 196                   # half-chunk for the split tail epilogues

_cache = {}


def _build():
    import concourse.bacc as bacc
    import concourse.mybir as mybir
    import concourse.tile as tile
    from concourse.masks import make_identity

    f32 = mybir.dt.float32
    f16 = mybir.dt.float16
    f8 = mybir.dt.float8e4
    AF = mybir.ActivationFunctionType
    ALU = mybir.AluOpType

    nc = bacc.Bacc(
        "TRN2",
        target_bir_lowering=False,
        debug=False,
        enable_asserts=False,
        num_devices=N_CORES,
    )

    x_ap = nc.dram_tensor("x", (NS, D, L), f32, kind="ExternalInput").ap()
    w_ap = nc.dram_tensor("weight", (K, D), f32, kind="ExternalInput").ap()
    s_ap = nc.dram_tensor("scale", (1,), f32, kind="ExternalInput").ap()
    o_ap = nc.dram_tensor("out", (NS, K, L), f8, kind="ExternalOutput").ap()

    def ch(a, b):  # cols covering chunks [a, b)
        return slice(a * LC, b * LC)

    # Q0 (SWDGE cast-on-load) transfer plan: graded sizes so PE starts
    # early and is then fed continuously; fine-grained interleaved
    # pieces for the second pair so each completion sem gates at most
    # two chunks of matmuls.
    stream = [
        (0, ch(0, 2)), (1, ch(0, 2)),
        (0, ch(2, 5)), (1, ch(2, 5)),
        (0, ch(5, 8)), (1, ch(5, 8)),
        (2, ch(0, 3)), (3, ch(0, 3)),
        (2, ch(3, 5)), (3, ch(3, 5)),
        (2, ch(5, 6)), (3, ch(5, 6)),
    ]

    with tile.TileContext(nc) as tc:
        with (
            tc.tile_pool(name="consts", bufs=1) as consts,
            tc.tile_pool(name="xin", bufs=4) as xpool,
            tc.tile_pool(name="xsq", bufs=4) as xqpool,
            tc.tile_pool(name="outp", bufs=2) as opool,
            tc.tile_pool(name="psum", bufs=4, space="PSUM") as pspool,
            tc.tile_pool(name="psum1", bufs=1, space="PSUM") as pspool1,
        ):
            xts = [
                xpool.tile([D, L], f16, tag="xt", name=f"x_{n}")
                for n in range(NS)
            ]
            xqs = [
                xqpool.tile([D, L], f16, tag="xq", name=f"xsq_{n}")
                for n in range(NS)
            ]

            # ---- input stream (SWDGE Q0, cast f32->f16 on load) ----------
            ident = consts.tile([K, K], f32)
            for i, (n, sl) in enumerate(stream):
                nc.gpsimd.dma_start(out=xts[n][:, sl], in_=x_ap[n][:, sl])
                if i == 0:
                    make_identity(nc, ident)

            # ---- HWDGE raw-f32 loads for the tail chunks of n2/n3 --------
            # These drain alongside the Q0 stream and land mid-stream with
            # negligible completion lag; ACT casts them to f16 and DVE
            # squares them straight from f32, so chunks 6-7 of the last
            # pair are fully compute-ready before the Q0 stream even ends.
            xfs = {}
            for n in (2, 3):
                xf = xpool.tile([D, 2 * LC], f32, tag="xf", name=f"xf_{n}")
                xfs[n] = xf
                nc.sync.dma_start(out=xf, in_=x_ap[n][:, ch(6, 8)])
            for n in (2, 3):
                nc.scalar.activation(
                    xts[n][:, ch(6, 8)], xfs[n], AF.Identity,
                )

            # ---- weight / scale prep (HWDGE, overlaps the stream) --------
            s_t = consts.tile([1, 1], f32)
            nc.sync.dma_start(out=s_t, in_=s_ap.to_broadcast((1, 1)))
            w2 = consts.tile([2 * K, D], f32)
            nc.sync.dma_start(out=w2[0:K, :], in_=w_ap)
            nc.sync.dma_start(out=w2[K : 2 * K, :], in_=w_ap)

            ones_row = consts.tile([1, 128], f32)
            nc.vector.memset(ones_row, 1.0)
            ones16 = consts.tile([D, K], f16)
            nc.vector.memset(ones16, 1.0)

            # broadcast scale to all 128 partitions via 1-col fp32 matmul
            ps_s = pspool1.tile([128, 1], f32, name="ps_s")
            nc.tensor.matmul(ps_s, ones_row, s_t, start=True, stop=True)
            s_b = consts.tile([128, 1], f32)
            nc.vector.tensor_scalar_mul(s_b, in0=ps_s, scalar1=1.0)
            inv_s2 = consts.tile([128, 1], f32)
            nc.vector.tensor_mul(inv_s2, s_b, s_b)
            nc.vector.reciprocal(inv_s2, inv_s2)

            w_sq = consts.tile([2 * K, D], f32)
            nc.vector.tensor_mul(w_sq, w2, w2)
            c_sq = consts.tile([2 * K, 1], f32)
            nc.vector.reduce_sum(out=c_sq, in_=w_sq, axis=mybir.AxisListType.X)
            c_sq_s = consts.tile([2 * K, 1], f32)
            nc.vector.tensor_mul(c_sq_s, c_sq, inv_s2)
            # fp8 offset encoding: store e4m3(dist - 2D/s^2); the host adds
            # the offset back. Centering kills the common mode so e4m3's
            # 6% relative step lands on the +-170 residual.
            bias2 = consts.tile([2 * K, 1], f32)
            nc.vector.tensor_scalar(
                out=bias2, in0=inv_s2,
                scalar1=-float(2 * D), scalar2=c_sq_s,
                op0=ALU.mult, op1=ALU.add,
            )

            ps_w = pspool1.tile([D, K], f32, name="ps_w")
            nc.tensor.transpose(ps_w, w2[0:K, :], ident)
            wT16 = consts.tile([D, K], f16)
            nc.vector.tensor_scalar_mul(wT16, in0=ps_w, scalar1=-2.0)

            # ---- derived stream: fp16 x^2 on DVE, in arrival order -------
            # (the HWDGE-loaded tail chunks are squared straight from f32,
            # ordered after the n0/n1 squares so DVE never stalls on them)
            for n, sl in stream[:6]:
                nc.vector.tensor_mul(xqs[n][:, sl], xts[n][:, sl], xts[n][:, sl])
            for n in (2, 3):
                nc.vector.tensor_mul(xqs[n][:, ch(6, 8)], xfs[n], xfs[n])
            for n, sl in stream[6:]:
                nc.vector.tensor_mul(xqs[n][:, sl], xts[n][:, sl], xts[n][:, sl])

            # ---- matmuls + epilogues + HWDGE output pieces ---------------
            for pair in range(NS // 2):
                n0, n1 = 2 * pair, 2 * pair + 1
                out_t = opool.tile([2 * K, L], f8, tag="out_t", name=f"out_{pair}")
                o_pair = o_ap[2 * pair : 2 * pair + 2].rearrange("a k l -> (a k) l")
                last_pair = pair == NS // 2 - 1
                # pair 1's chunks are emitted in data-arrival order: the
                # HWDGE-fed chunks 6-7 are ready mid-stream, well before
                # the Q0-gated chunks; PE executes its queue in order.
                chunk_order = [6, 7, 0, 1, 2, 3, 4, 5] if last_pair else range(8)
                for c in chunk_order:
                    sl = ch(c, c + 1)
                    ps = pspool.tile([2 * K, LC], f32, name="ps")
                    nc.tensor.matmul(
                        ps[0:K, :], wT16, xts[n0][:, sl],
                        start=True, stop=False, tile_position=(0, 0),
                    )
                    nc.tensor.matmul(
                        ps[K : 2 * K, :], wT16, xts[n1][:, sl],
                        start=True, stop=False, tile_position=(0, 64),
                    )
                    nc.tensor.matmul(
                        ps[0:K, :], ones16, xqs[n0][:, sl],
                        start=False, stop=True, tile_position=(0, 0),
                    )
                    nc.tensor.matmul(
                        ps[K : 2 * K, :], ones16, xqs[n1][:, sl],
                        start=False, stop=True, tile_position=(0, 64),
                    )
                    if last_pair and c in (4, 5):
                        # split the late epilogues ACT || DVE so each clears
                        # in ~0.35us instead of ~0.7us (and ACT's backlog
                        # doesn't stack onto the final chunk)
                        nc.scalar.activation(
                            out_t[:, c * LC : c * LC + HC],
                            ps[:, 0:HC], AF.Identity,
                            bias=bias2, scale=inv_s2,
                        )
                        nc.vector.tensor_scalar(
                            out=out_t[:, c * LC + HC : (c + 1) * LC],
                            in0=ps[:, HC:LC],
                            scalar1=inv_s2, scalar2=bias2,
                            op0=ALU.mult, op1=ALU.add,
                        )
                    else:
                        nc.scalar.activation(
                            out_t[:, sl], ps, AF.Identity,
                            bias=bias2, scale=inv_s2,
                        )
                    # ship finished columns on HWDGE: pair 0 as one full-L
                    # write; pair 1 in pieces ordered by readiness, so the
                    # last-ready piece is the final 50 KB chunk ch(5,6).
                    if not last_pair:
                        if c == 7:
                            nc.sync.dma_start(out=o_pair, in_=out_t)
                    elif c == 7:
                        es = ch(6, 8)
                        nc.sync.dma_start(out=o_pair[:, es], in_=out_t[:, es])
                    elif c == 2:
                        hs = ch(0, 3)
                        nc.sync.dma_start(out=o_pair[:, hs], in_=out_t[:, hs])
                    elif c == 4:
                        qs = ch(3, 5)
                        nc.sync.dma_start(out=o_pair[:, qs], in_=out_t[:, qs])
                    elif c == 5:
                        fs = ch(5, 6)
                        nc.sync.dma_start(out=o_pair[:, fs], in_=out_t[:, fs])

    nc.compile()
    return nc


def _get_nc():
    if "nc" not in _cache:
        _cache["nc"] = _build()
    return _cache["nc"]


def run(x, weight, scale, trace=False, tmpdir=None):
    from concourse.bass_utils import run_bass_kernel_spmd

    x = np.ascontiguousarray(np.asarray(x, dtype=np.float32))
    weight = np.ascontiguousarray(np.asarray(weight, dtype=np.float32))
    scale = np.ascontiguousarray(np.asarray(scale, dtype=np.float32))
    assert x.shape == (N, D, L) and weight.shape == (K, D) and scale.shape == (1,)

    nc = _get_nc()
    in_maps = [
        {"x": x[c * NS : (c + 1) * NS], "weight": weight, "scale": scale}
        for c in range(N_CORES)
    ]
    res = run_bass_kernel_spmd(
        nc, in_maps, core_ids=list(range(N_CORES)), trace=trace, tmpdir=tmpdir
    )
    out = np.concatenate([r["out"] for r in res.results], axis=0).astype(np.float32)
    out += np.float32(2.0 * D) / np.float32(scale[0] ** 2)
    return out, res


def kernel(x, weight, scale):
    out, _ = run(x, weight, scale, trace=False)
    return out


# revision 49
# speedup vs baseline: 1.1979x; 1.1979x over previous
"""VQ codebook squared-distance kernel for Trainium2 (8 NeuronCores).

Computes dist[n,k,l] = (||x[n,:,l]||^2 + ||w[k,:]||^2 - 2*x[n,:,l].w[k,:]) / scale^2
for x (32,128,3136) f32, weight (64,128) f32, scale (1,) f32 -> out (32,64,3136) f32.

Sharding: data-parallel over N (4 per core); weight/scale replicated.

Design (best-measured configuration across 12 HW iterations: 36445 /
37040 ns vs the 37038 ns baseline):
  - Input stream is HBM-stack-roofline-bound (~343 GB/s/core with both
    NCs of a stack active): 6.42 MB f32 x read in ~18us. The kernel is
    a saturated multi-resource equilibrium: PE ~13-16us, ACT ~11-14us,
    DVE ~13us busy inside a ~22us window, plus a FIXED ~8.5us NEFF end
    block (barrier + 257 walrus per-semaphore clears, invariant to
    kernel content and not HAM-gated). The back half runs as a conveyor
    paced by ACT's epilogue rate (~480ns fixed cost per ACT
    instruction) through PSUM-pool recycling; shifting epilogue work
    between ACT and DVE moves the pacer but not the total (measured
    both ways).
  - x loads via SWDGE Q0 cast-on-load f32->f16 in graded pieces:
    small head (PE starts by ~11us), fine interleaved tail so each
    completion sem gates at most two chunks of matmuls. SWDGE
    straggler: one SDMA engine lags the other 15 by an amount that
    grows with Q0 descriptor pressure (~0 at 8 transfers, ~0.4-1.65us
    at 12, ~2.4-4us beyond, or with 3+ full-image transfers).
  - Chunks 6-7 of n2/n3 arrive via HWDGE (raw f32, ACT-cast to f16,
    DVE-squared straight from f32): lag-free sems, ready mid-stream.
    Never put casts on GpSimd - Pool tensor ops lock the shared
    DVE/GpSimd SBUF ports and knock DVE out of 2x perf mode.
  - Outputs ride HWDGE in readiness-ordered pieces (no Q0 descriptor
    traffic, no queueing behind the straggler): pair 0 as one full-L
    write, pair 1 as ch(6,8)/ch(0,3)/ch(3,5)/ch(5,6) with the final
    50 KB piece last. Epilogues split ACT || DVE for the last chunks
    (NOTE: the DVE half is still serialized after the ACT half by
    cross-engine write ordering on the shared out_t tile; a separate
    single-writer tile avoids that but measured no net win - the
    conveyor just finds another pacer).
  - Output is offset fp8: e4m3(dist - 2D/s^2), host adds the offset
    back. Centering removes the ~2D/s^2 common mode so e4m3's ~6%
    step applies to the +-170 residual only: rel_l2 ~3e-3 (vs 2e-2
    budget) for half the write traffic.
  - PE: psum = (-2Wt)f16 @ x_f16 + ones_f16 @ (x^2)_f16, two images
    per PSUM tile via column tiling (tile_position (0,0)/(0,64));
    the h0/h64 col-group matmul pairs run concurrently (~1.4ns/col).
  - scale broadcast 1->128 via 1-col fp32 matmul; weight transpose on
    PE (identity built early on gpsimd).
"""

import numpy as np

N, D, L, K = 32, 128, 3136, 64
N_CORES = 8
NS = N // N_CORES          # n's per core
LC = 392                   # matmul chunk (8 per image, one PSUM bank)
HC = 196                   # half-chunk for the split tail epilogues

_cache = {}


def _build():
    import concourse.bacc as bacc
    import concourse.mybir as mybir
    import concourse.tile as tile
    from concourse.masks import make_identity

    f32 = mybir.dt.float32
    f16 = mybir.dt.float16
    f8 = mybir.dt.float8e4
    AF = mybir.ActivationFunctionType
    ALU = mybir.AluOpType

    nc = bacc.Bacc(
        "TRN2",
        target_bir_lowering=False,
        debug=False,
        enable_asserts=False,
        num_devices=N_CORES,
    )

    x_ap = nc.dram_tensor("x", (NS, D, L), f32, kind="ExternalInput").ap()
    w_ap = nc.dram_tensor("weight", (K, D), f32, kind="ExternalInput").ap()
    s_ap = nc.dram_tensor("scale", (1,), f32, kind="ExternalInput").ap()
    o_ap = nc.dram_tensor("out", (NS, K, L), f8, kind="ExternalOutput").ap()

    def ch(a, b):  # cols covering chunks [a, b)
        return slice(a * LC, b * LC)

    # Q0 (SWDGE cast-on-load) transfer plan: graded sizes so PE starts
    # early and is then fed continuously; fine-grained interleaved
    # pieces for the second pair so each completion sem gates at most
    # two chunks of matmuls.
    stream = [
        (0, ch(0, 2)), (1, ch(0, 2)),
        (0, ch(2, 5)), (1, ch(2, 5)),
        (0, ch(5, 8)), (1, ch(5, 8)),
        (2, ch(0, 3)), (3, ch(0, 3)),
        (2, ch(3, 5)), (3, ch(3, 5)),
        (2, ch(5, 6)), (3, ch(5, 6)),
    ]

    with tile.TileContext(nc) as tc:
        with (
            tc.tile_pool(name="consts", bufs=1) as consts,
            tc.tile_pool(name="xin", bufs=4) as xpool,
            tc.tile_pool(name="xsq", bufs=4) as xqpool,
            tc.tile_pool(name="outp", bufs=2) as opool,
            tc.tile_pool(name="psum", bufs=3, space="PSUM") as pspool,
            tc.tile_pool(name="psum1", bufs=1, space="PSUM") as pspool1,
        ):
            xts = [
                xpool.tile([D, L], f16, tag="xt", name=f"x_{n}")
                for n in range(NS)
            ]
            xqs = [
                xqpool.tile([D, L], f16, tag="xq", name=f"xsq_{n}")
                for n in range(NS)
            ]

            # ---- input stream (SWDGE Q0, cast f32->f16 on load) ----------
            ident = consts.tile([K, K], f32)
            for i, (n, sl) in enumerate(stream):
                nc.gpsimd.dma_start(out=xts[n][:, sl], in_=x_ap[n][:, sl])
                if i == 0:
                    make_identity(nc, ident)

            # ---- HWDGE raw-f32 loads for the tail chunks of n2/n3 --------
            # These drain alongside the Q0 stream and land mid-stream with
            # negligible completion lag; ACT casts them to f16 and DVE
            # squares them straight from f32, so chunks 6-7 of the last
            # pair are fully compute-ready before the Q0 stream even ends.
            xfs = {}
            for n in (2, 3):
                xf = xpool.tile([D, 2 * LC], f32, tag="xf", name=f"xf_{n}")
                xfs[n] = xf
                nc.sync.dma_start(out=xf, in_=x_ap[n][:, ch(6, 8)])
            for n in (2, 3):
                nc.scalar.activation(
                    xts[n][:, ch(6, 8)], xfs[n], AF.Identity,
                )

            # ---- weight / scale prep (HWDGE, overlaps the stream) --------
            s_t = consts.tile([1, 1], f32)
            nc.sync.dma_start(out=s_t, in_=s_ap.to_broadcast((1, 1)))
            w2 = consts.tile([2 * K, D], f32)
            nc.sync.dma_start(out=w2[0:K, :], in_=w_ap)
            nc.sync.dma_start(out=w2[K : 2 * K, :], in_=w_ap)

            ones_row = consts.tile([1, 128], f32)
            nc.vector.memset(ones_row, 1.0)
            ones16 = consts.tile([D, K], f16)
            nc.vector.memset(ones16, 1.0)

            # broadcast scale to all 128 partitions via 1-col fp32 matmul
            ps_s = pspool1.tile([128, 1], f32, name="ps_s")
            nc.tensor.matmul(ps_s, ones_row, s_t, start=True, stop=True)
            s_b = consts.tile([128, 1], f32)
            nc.vector.tensor_scalar_mul(s_b, in0=ps_s, scalar1=1.0)
            inv_s2 = consts.tile([128, 1], f32)
            nc.vector.tensor_mul(inv_s2, s_b, s_b)
            nc.vector.reciprocal(inv_s2, inv_s2)

            w_sq = consts.tile([2 * K, D], f32)
            nc.vector.tensor_mul(w_sq, w2, w2)
            c_sq = consts.tile([2 * K, 1], f32)
            nc.vector.reduce_sum(out=c_sq, in_=w_sq, axis=mybir.AxisListType.X)
            c_sq_s = consts.tile([2 * K, 1], f32)
            nc.vector.tensor_mul(c_sq_s, c_sq, inv_s2)
            # fp8 offset encoding: store e4m3(dist - 2D/s^2); the host adds
            # the offset back. Centering kills the common mode so e4m3's
            # 6% relative step lands on the +-170 residual.
            bias2 = consts.tile([2 * K, 1], f32)
            nc.vector.tensor_scalar(
                out=bias2, in0=inv_s2,
                scalar1=-float(2 * D), scalar2=c_sq_s,
                op0=ALU.mult, op1=ALU.add,
            )

            ps_w = pspool1.tile([D, K], f32, name="ps_w")
            nc.tensor.transpose(ps_w, w2[0:K, :], ident)
            wT16 = consts.tile([D, K], f16)
            nc.vector.tensor_scalar_mul(wT16, in0=ps_w, scalar1=-2.0)

            # ---- derived stream: fp16 x^2 on DVE, in arrival order -------
            # (the HWDGE-loaded tail chunks are squared straight from f32,
            # ordered after the n0/n1 squares so DVE never stalls on them)
            for n, sl in stream[:6]:
                nc.vector.tensor_mul(xqs[n][:, sl], xts[n][:, sl], xts[n][:, sl])
            for n in (2, 3):
                nc.vector.tensor_mul(xqs[n][:, ch(6, 8)], xfs[n], xfs[n])
            for n, sl in stream[6:]:
                nc.vector.tensor_mul(xqs[n][:, sl], xts[n][:, sl], xts[n][:, sl])

            # ---- matmuls + merged epilogues + HWDGE output pieces --------
            # Two chunks share one 2-bank PSUM tile ([128, 1024] f32:
            # chunk A at cols 0:392 in bank 0, chunk B at cols 512:904 in
            # bank 1), and ONE ACT epilogue covers both via a strided
            # [128, 2, 392] read. ACT's ~480ns fixed cost per instruction
            # paces the back-half conveyor, so halving the epilogue
            # instruction count cuts ACT's saturated-phase work ~11->7.5us.
            def mm_quad(ps, cols, n0, n1, c):
                sl = ch(c, c + 1)
                nc.tensor.matmul(
                    ps[0:K, cols], wT16, xts[n0][:, sl],
                    start=True, stop=False, tile_position=(0, 0),
                )
                nc.tensor.matmul(
                    ps[K : 2 * K, cols], wT16, xts[n1][:, sl],
                    start=True, stop=False, tile_position=(0, 64),
                )
                nc.tensor.matmul(
                    ps[0:K, cols], ones16, xqs[n0][:, sl],
                    start=False, stop=True, tile_position=(0, 0),
                )
                nc.tensor.matmul(
                    ps[K : 2 * K, cols], ones16, xqs[n1][:, sl],
                    start=False, stop=True, tile_position=(0, 64),
                )

            for pair in range(NS // 2):
                n0, n1 = 2 * pair, 2 * pair + 1
                out_t = opool.tile([2 * K, L], f8, tag="out_t", name=f"out_{pair}")
                o_pair = o_ap[2 * pair : 2 * pair + 2].rearrange("a k l -> (a k) l")
                last_pair = pair == NS // 2 - 1
                # pair 1's chunk pairs run in data-arrival order: the
                # HWDGE-fed chunks 6-7 are ready mid-stream, well before
                # the Q0-gated ones; PE executes its queue in order.
                cpairs = [(6, 7), (0, 1), (2, 3), (4, 5)] if last_pair \
                    else [(0, 1), (2, 3), (4, 5), (6, 7)]
                for a, b in cpairs:
                    ps = pspool.tile([2 * K, 1024], f32, name="ps")
                    mm_quad(ps, slice(0, LC), n0, n1, a)
                    mm_quad(ps, slice(512, 512 + LC), n0, n1, b)
                    src = ps[:, 0:1024].rearrange(
                        "p (two c) -> p two c", two=2
                    )[:, :, 0:LC]
                    nc.scalar.activation(
                        out_t[:, ch(a, b + 1)], src, AF.Identity,
                        bias=bias2, scale=inv_s2,
                    )
                    # ship finished columns on HWDGE: pair 0 as one full-L
                    # write; pair 1 in pieces ordered by readiness, so the
                    # last-ready piece is the final 100 KB ch(4,6).
                    if not last_pair:
                        if a == 6:
                            nc.sync.dma_start(out=o_pair, in_=out_t)
                    else:
                        pc = ch(a, b + 1)
                        nc.sync.dma_start(out=o_pair[:, pc], in_=out_t[:, pc])

    nc.compile()
    return nc


def _get_nc():
    if "nc" not in _cache:
        _cache["nc"] = _build()
    return _cache["nc"]


def run(x, weight, scale, trace=False, tmpdir=None):
    from concourse.bass_utils import run_bass_kernel_spmd

    x = np.ascontiguousarray(np.asarray(x, dtype=np.float32))
    weight = np.ascontiguousarray(np.asarray(weight, dtype=np.float32))
    scale = np.ascontiguousarray(np.asarray(scale, dtype=np.float32))
    assert x.shape == (N, D, L) and weight.shape == (K, D) and scale.shape == (1,)

    nc = _get_nc()
    in_maps = [
        {"x": x[c * NS : (c + 1) * NS], "weight": weight, "scale": scale}
        for c in range(N_CORES)
    ]
    res = run_bass_kernel_spmd(
        nc, in_maps, core_ids=list(range(N_CORES)), trace=trace, tmpdir=tmpdir
    )
    out = np.concatenate([r["out"] for r in res.results], axis=0).astype(np.float32)
    out += np.float32(2.0 * D) / np.float32(scale[0] ** 2)
    return out, res


def kernel(x, weight, scale):
    out, _ = run(x, weight, scale, trace=False)
    return out
